# revision 1
# baseline (speedup 1.0000x reference)
"""Trainium2 Bass kernel for nn_CrossAttention3D (cross-attention transformer block).

Strategy (8 NeuronCores, SPMD, no collectives):
  - Shard the 8192 query tokens across cores (1024 q-tokens/core). K/V work
    (2048 tokens) is replicated per core. Attention, projections, MLP and all
    LayerNorms shard cleanly along q-tokens.
  - Layouts: activations kept feature-major (channels on partitions, tokens on
    free axis) which matches the channel-major DRAM layout of q/k/v; vh kept
    token-major; scores computed transposed (S^T: k on partitions, q free) so
    softmax needs only exp (no max-subtract: scores are O(1)); the softmax
    denominator is produced by a ones-column appended to vh in the AV matmul.
  - LayerNorm means are folded into the projection weights on the host
    (W'' = diag(g)W - (1/512) ones (g W)^T); per-token rstd is applied
    post-matmul via ones-matmul row stats + rank-1 broadcast matmuls.
  - dtypes: float32r for most matmuls (full PE rate, ~1e-4 rounding); bf16 for
    the attention-probability x V matmul and the MLP matmuls (SBUF pressure).
"""

import sys, os

for _p in ("/opt/trn_rl_repo", "/root/.axon_site/_ro/trn_rl_repo"):
    if os.path.isdir(_p) and _p not in sys.path:
        sys.path.insert(0, _p)

import numpy as np
from contextlib import ExitStack

import concourse.bacc as bacc
import concourse.tile as tile
from concourse import mybir
from concourse.bass_utils import run_bass_kernel_spmd

F32R = mybir.dt.float32r
F32 = mybir.dt.float32
BF16 = mybir.dt.bfloat16
AF = mybir.ActivationFunctionType
OP = mybir.AluOpType

DIM = 512
HEADS = 8
DH = 64
EPS = 1e-5
NCORES = 8
QTOK = 32 * 32 * 8          # 8192
KTOK = 16 * 16 * 8          # 2048
QPC = QTOK // NCORES        # 1024 q tokens per core
CT = DIM // 128             # 4 channel tiles
QB = QPC // 512             # 2 q blocks per core
KBLK = KTOK // 512          # 4 kv token blocks
KTT = KTOK // 128           # 16 kv token tiles

_CACHE = {}


def _build_program():
    nc = bacc.Bacc(None, target_bir_lowering=False)

    d = {}
    def dram(name, shape, dt, out=False):
        d[name] = nc.dram_tensor(name, shape, dt, kind="ExternalOutput" if out else "ExternalInput").ap()
        return d[name]

    dram("qT", [DIM, QPC], F32R)
    dram("kT", [DIM, KTOK], F32R)
    dram("vT", [DIM, KTOK], F32R)
    dram("Wq", [DIM, DIM], F32R)
    dram("Wk", [DIM, DIM], F32R)
    dram("Wv", [DIM, DIM], F32R)
    dram("Wp", [DIM, DIM], F32R)
    dram("W1", [DIM, 2 * DIM], BF16)
    dram("W2", [2 * DIM, DIM], BF16)
    dram("cpack", [128, 41], F32)     # packed per-partition constants, see host
    dram("bvr", [1, DIM], F32)        # v-projection bias as a row
    dram("vones", [1, HEADS], BF16)   # ones for the denominator column of vh
    dram("onesc", [128, 1], F32R)
    dram("onesr", [1, 128], F32R)
    dram("out", [DIM, QPC], F32, out=True)

    # cpack column map
    C_BQ, C_BK, C_BP, C_B1, C_B2 = 0, 4, 8, 12, 20
    C_GPRE, C_BPRE, C_GPOST, C_BPOST, C_EPS = 24, 28, 32, 36, 40

    with tile.TileContext(nc) as tc, ExitStack() as ctx:
        ctx.enter_context(nc.allow_low_precision(reason="float32r/bf16 rounding is intentional"))
        consts = ctx.enter_context(tc.tile_pool(name="consts", bufs=1))
        wpool = ctx.enter_context(tc.tile_pool(name="wpool", bufs=1))
        wqkvp = ctx.enter_context(tc.tile_pool(name="wqkvp", bufs=4))
        rawp = ctx.enter_context(tc.tile_pool(name="rawp", bufs=8))
        khp = ctx.enter_context(tc.tile_pool(name="khp", bufs=1))
        qhp = ctx.enter_context(tc.tile_pool(name="qhp", bufs=8))
        vhp = ctx.enter_context(tc.tile_pool(name="vhp", bufs=1))
        atp = ctx.enter_context(tc.tile_pool(name="atp", bufs=4))
        ztp = ctx.enter_context(tc.tile_pool(name="ztp", bufs=4))
        zlnp = ctx.enter_context(tc.tile_pool(name="zlnp", bufs=4))
        zlbp = ctx.enter_context(tc.tile_pool(name="zlbp", bufs=4))
        h1p = ctx.enter_context(tc.tile_pool(name="h1p", bufs=8))
        resp = ctx.enter_context(tc.tile_pool(name="resp", bufs=4))
        outp = ctx.enter_context(tc.tile_pool(name="outp", bufs=2))
        sqp = ctx.enter_context(tc.tile_pool(name="sqp", bufs=3))
        ptmpp = ctx.enter_context(tc.tile_pool(name="ptmpp", bufs=2))
        rbcp = ctx.enter_context(tc.tile_pool(name="rbcp", bufs=2))
        ptp = ctx.enter_context(tc.tile_pool(name="ptp", bufs=4))
        lntp = ctx.enter_context(tc.tile_pool(name="lntp", bufs=2))
        rows = ctx.enter_context(tc.tile_pool(name="rows", bufs=1))

        cpk = consts.tile([128, 41], F32)
        nc.sync.dma_start(out=cpk[:], in_=d["cpack"][:])
        bvc = consts.tile([128, DIM], F32)
        nc.sync.dma_start(out=bvc[:], in_=d["bvr"].to_broadcast([128, DIM]))
        onesc = consts.tile([128, 1], F32R)
        nc.sync.dma_start(out=onesc[:], in_=d["onesc"][:])
        onesr = consts.tile([1, 128], F32R)
        nc.sync.dma_start(out=onesr[:], in_=d["onesr"][:])
        vob = consts.tile([1, HEADS], BF16)
        nc.sync.dma_start(out=vob[:], in_=d["vones"][:])

        def ccol(idx):
            return cpk[:, idx:idx + 1]

        # resident weights: Wp f32r; W1/W2 bf16
        wp = [wpool.tile([128, DIM], F32R, tag=f"wp{i}", name=f"wp{i}") for i in range(CT)]
        w1 = [wpool.tile([128, 2 * DIM], BF16, tag=f"w1{i}", name=f"w1{i}") for i in range(CT)]
        w2 = [wpool.tile([128, DIM], BF16, tag=f"w2{i}", name=f"w2{i}") for i in range(2 * CT)]
        for i in range(CT):
            nc.sync.dma_start(out=wp[i][:], in_=d["Wp"][128 * i:128 * (i + 1), :])
            nc.sync.dma_start(out=w1[i][:], in_=d["W1"][128 * i:128 * (i + 1), :])
        for i in range(2 * CT):
            nc.sync.dma_start(out=w2[i][:], in_=d["W2"][128 * i:128 * (i + 1), :])

        # ---------------- phase 1: q/k/v LN-stats + projections ----------------
        p1 = ExitStack()
        p1proj = p1.enter_context(tc.tile_pool(name="p1proj", bufs=4, space="PSUM"))
        p1stat = p1.enter_context(tc.tile_pool(name="p1stat", bufs=2, space="PSUM"))
        p1bc = p1.enter_context(tc.tile_pool(name="p1bc", bufs=1, space="PSUM"))
        p1col = p1.enter_context(tc.tile_pool(name="p1col", bufs=1, space="PSUM"))

        def load_w3(dname):
            ws = []
            for i in range(CT):
                w = wqkvp.tile([128, DIM], F32R, tag="wqkv", name=f"{dname}_{i}")
                nc.sync.dma_start(out=w[:], in_=d[dname][128 * i:128 * (i + 1), :])
                ws.append(w)
            return ws

        def load_raw_block(dname, blk):
            ts = []
            for i in range(CT):
                t = rawp.tile([128, 512], F32R, tag="rawblk", name=f"{dname}r{blk}_{i}")
                nc.sync.dma_start(out=t[:], in_=d[dname][128 * i:128 * (i + 1), 512 * blk:512 * (blk + 1)])
                ts.append(t)
            return ts

        def block_rstd_row(x_tiles, stat_pool):
            """(mu_row, rstd_row) (1,512) f32r rows for a 512-token block (4 ch tiles)."""
            ps_mu = stat_pool.tile([1, 512], F32, tag="p1rows" if stat_pool is p1stat else "trow")
            for i in range(CT):
                nc.tensor.matmul(ps_mu[:], lhsT=onesc[:], rhs=x_tiles[i][:],
                                 start=(i == 0), stop=(i == CT - 1))
            mu_row = rows.tile([1, 512], F32R, tag="murow")
            nc.scalar.activation(mu_row[:], ps_mu[:], AF.Copy, scale=1.0 / DIM)
            ps_sq = stat_pool.tile([1, 512], F32, tag="p1rows" if stat_pool is p1stat else "trow")
            for i in range(CT):
                sq = sqp.tile([128, 512], F32R, tag="sq")
                nc.vector.tensor_tensor(out=sq[:], in0=x_tiles[i][:], in1=x_tiles[i][:], op=OP.mult)
                nc.tensor.matmul(ps_sq[:], lhsT=onesc[:], rhs=sq[:],
                                 start=(i == 0), stop=(i == CT - 1))
            msq_row = rows.tile([1, 512], F32R, tag="msqrow")
            nc.scalar.activation(msq_row[:], ps_sq[:], AF.Copy, scale=1.0 / DIM)
            tmp_row = rows.tile([1, 512], F32R, tag="tmprow")
            nc.vector.tensor_tensor(out=tmp_row[:], in0=mu_row[:], in1=mu_row[:], op=OP.mult)
            nc.vector.tensor_tensor(out=msq_row[:], in0=msq_row[:], in1=tmp_row[:], op=OP.subtract)
            nc.scalar.activation(tmp_row[:], msq_row[:], AF.Sqrt, bias=cpk[0:1, C_EPS:C_EPS + 1])
            rstd_row = rows.tile([1, 512], F32R, tag="rstdrow")
            nc.vector.reciprocal(out=rstd_row[:], in_=tmp_row[:])
            return mu_row, rstd_row

        def bcast_sb(row, ps_pool, ps_tag, nparts=128):
            ps_b = ps_pool.tile([128, 512], F32, tag=ps_tag)
            nc.tensor.matmul(ps_b[:], lhsT=onesr[:], rhs=row[:], start=True, stop=True)
            rsb = rbcp.tile([128, 512], F32, tag="rbc")
            nc.vector.tensor_copy(rsb[0:nparts, :], ps_b[0:nparts, :])
            return rsb

        # K: stats + projection -> khT (feature-major, 4 x (128, 2048))
        wk = load_w3("Wk")
        khT = [khp.tile([128, KTOK], F32R, tag=f"kh{i}", name=f"kh{i}") for i in range(CT)]
        for blk in range(KBLK):
            kraw = load_raw_block("kT", blk)
            _, rrow = block_rstd_row(kraw, p1stat)
            rsb = bcast_sb(rrow, p1bc, "p1bc")
            sl = slice(512 * blk, 512 * (blk + 1))
            for ot in range(CT):
                ps = p1proj.tile([128, 512], F32, tag="p1proj")
                for ci in range(CT):
                    nc.tensor.matmul(ps[:], lhsT=wk[ci][:, 128 * ot:128 * (ot + 1)],
                                     rhs=kraw[ci][:], start=(ci == 0), stop=(ci == CT - 1))
                tmp = ptmpp.tile([128, 512], F32R, tag="ptmp")
                nc.vector.tensor_tensor(out=tmp[:], in0=ps[:], in1=rsb[:], op=OP.mult)
                nc.vector.tensor_scalar_add(khT[ot][:, sl], tmp[:], ccol(C_BK + ot))

        # V: stats + projection -> vh_aug (token-major, 16 x (128, 8*65) bf16)
        wv = load_w3("Wv")
        vh = [vhp.tile([128, HEADS * (DH + 1)], BF16, tag=f"vh{j}", name=f"vh{j}") for j in range(KTT)]
        for j in range(KTT):
            nc.sync.dma_start(out=vh[j][:].rearrange("p (h e) -> p h e", e=DH + 1)[:, :, DH:DH + 1],
                              in_=d["vones"].to_broadcast([128, HEADS]).unsqueeze(2))
        for blk in range(KBLK):
            vraw = load_raw_block("vT", blk)
            _, rrow = block_rstd_row(vraw, p1stat)
            # rstd as columns for the 4 token tiles of this block (bf16: the
            # fp32r matmul path rejects tiny moving dims)
            rbf = rows.tile([1, 512], BF16, tag="rstdbf")
            nc.vector.tensor_copy(rbf[:], rrow[:])
            pcols = p1col.tile([128, 4], F32, tag="p1col")
            for jj in range(4):
                nc.tensor.matmul(pcols[:, jj:jj + 1], lhsT=rbf[0:1, 128 * jj:128 * (jj + 1)],
                                 rhs=vob[0:1, 0:1], start=True, stop=True)
            rcols = rbcp.tile([128, 4], F32, tag="rcols")
            nc.vector.tensor_copy(rcols[:], pcols[:])
            for jj in range(4):
                j = 4 * blk + jj
                ps = p1proj.tile([128, 512], F32, tag="p1proj")
                for ci in range(CT):
                    nc.tensor.matmul(ps[:], lhsT=vraw[ci][:, 128 * jj:128 * (jj + 1)],
                                     rhs=wv[ci][:], start=(ci == 0), stop=(ci == CT - 1))
                tmp = ptmpp.tile([128, 512], F32, tag="ptmp")
                nc.vector.tensor_scalar_mul(tmp[:], ps[:], rcols[:, jj:jj + 1])
                vdst = vh[j][:].rearrange("p (h e) -> p h e", e=DH + 1)[:, :, 0:DH]
                nc.vector.tensor_tensor(out=vdst, in0=tmp[:].rearrange("p (h dh) -> p h dh", dh=DH),
                                        in1=bvc[:].rearrange("p (h dh) -> p h dh", dh=DH), op=OP.add)

        # Q: stats + projection -> qhT (feature-major, per qb: 4 x (128, 512))
        wq = load_w3("Wq")
        qhT = {}
        for qb in range(QB):
            qraw = load_raw_block("qT", qb)
            _, rrow = block_rstd_row(qraw, p1stat)
            rsb = bcast_sb(rrow, p1bc, "p1bc")
            for ot in range(CT):
                ps = p1proj.tile([128, 512], F32, tag="p1proj")
                for ci in range(CT):
                    nc.tensor.matmul(ps[:], lhsT=wq[ci][:, 128 * ot:128 * (ot + 1)],
                                     rhs=qraw[ci][:], start=(ci == 0), stop=(ci == CT - 1))
                qt = qhp.tile([128, 512], F32R, tag="qh")
                tmp = ptmpp.tile([128, 512], F32R, tag="ptmp")
                nc.vector.tensor_tensor(out=tmp[:], in0=ps[:], in1=rsb[:], op=OP.mult)
                nc.vector.tensor_scalar_add(qt[:], tmp[:], ccol(C_BQ + ot))
                qhT[(qb, ot)] = qt

        p1.close()

        # ---------------- phase 2: attention + per-qb tail ----------------
        p2 = ExitStack()
        psS = p2.enter_context(tc.tile_pool(name="psS", bufs=3, space="PSUM"))
        psAV = p2.enter_context(tc.tile_pool(name="psAV", bufs=2, space="PSUM"))
        tproj = p2.enter_context(tc.tile_pool(name="tproj", bufs=1, space="PSUM"))
        trows = p2.enter_context(tc.tile_pool(name="trows", bufs=2, space="PSUM"))

        def tail_stats(x_tiles):
            mu_row, rstd_row = block_rstd_row(x_tiles, trows)
            mu_bc = bcast_sb(mu_row, psS, "s")
            rstd_bc = bcast_sb(rstd_row, psS, "s")
            return mu_bc, rstd_bc

        for qb in range(QB):
            # --- attention for this q block ---
            aT = [atp.tile([128, 512], F32R, tag="aT", name=f"aT{qb}_{i}") for i in range(CT)]
            for hp in range(CT):           # head pair = feature tile of qh/kh
                ps_avA = psAV.tile([DH + 1, 512], F32, tag="av")
                ps_avB = psAV.tile([DH + 1, 512], F32, tag="av")
                for kt in range(KTT):
                    ps_a = psS.tile([128, 512], F32, tag="s")
                    ps_b = psS.tile([128, 512], F32, tag="s")
                    nc.tensor.matmul(ps_a[:], lhsT=khT[hp][0:64, 128 * kt:128 * (kt + 1)],
                                     rhs=qhT[(qb, hp)][0:64, :], start=True, stop=True,
                                     tile_position=(0, 0))
                    nc.tensor.matmul(ps_b[:], lhsT=khT[hp][64:128, 128 * kt:128 * (kt + 1)],
                                     rhs=qhT[(qb, hp)][64:128, :], start=True, stop=True,
                                     tile_position=(64, 0))
                    pa = ptp.tile([128, 512], BF16, tag="pt")
                    pb = ptp.tile([128, 512], BF16, tag="pt")
                    nc.scalar.activation(pa[:], ps_a[:], AF.Exp)
                    nc.scalar.activation(pb[:], ps_b[:], AF.Exp)
                    hA, hB = 2 * hp, 2 * hp + 1
                    nc.tensor.matmul(ps_avA[:], lhsT=vh[kt][:, (DH + 1) * hA:(DH + 1) * (hA + 1)],
                                     rhs=pa[:], start=(kt == 0), stop=(kt == KTT - 1))
                    nc.tensor.matmul(ps_avB[:], lhsT=vh[kt][:, (DH + 1) * hB:(DH + 1) * (hB + 1)],
                                     rhs=pb[:], start=(kt == 0), stop=(kt == KTT - 1))
                # normalize by the denominator row (row DH), write into aT[hp]
                for half, ps_av in ((0, ps_avA), (1, ps_avB)):
                    rec = rows.tile([1, 512], F32R, tag="recrow")
                    nc.vector.reciprocal(out=rec[:], in_=ps_av[DH:DH + 1, :])
                    rb_sb = bcast_sb(rec, psS, "s", nparts=64)
                    nc.vector.tensor_tensor(out=aT[hp][64 * half:64 * (half + 1), :],
                                            in0=ps_av[0:DH, :], in1=rb_sb[0:64, :], op=OP.mult)

            # --- tail: out-projection, ln_pre, MLP, residual, ln_post, store ---
            zt = []
            for ot in range(CT):
                ps = tproj.tile([128, 512], F32, tag="tproj")
                for ci in range(CT):
                    nc.tensor.matmul(ps[:], lhsT=wp[ci][:, 128 * ot:128 * (ot + 1)],
                                     rhs=aT[ci][:], start=(ci == 0), stop=(ci == CT - 1))
                z = ztp.tile([128, 512], F32R, tag="zt")
                nc.vector.tensor_scalar_add(z[:], ps[:], ccol(C_BP + ot))
                zt.append(z)
            mu_bc, rstd_bc = tail_stats(zt)
            zln, zlb = [], []
            for ot in range(CT):
                t1 = lntp.tile([128, 512], F32, tag="lnt")
                nc.vector.tensor_tensor(out=t1[:], in0=zt[ot][:], in1=mu_bc[:], op=OP.subtract)
                t2 = lntp.tile([128, 512], F32, tag="lnt2")
                nc.vector.tensor_tensor(out=t2[:], in0=t1[:], in1=rstd_bc[:], op=OP.mult)
                zl = zlnp.tile([128, 512], F32R, tag="zln")
                nc.vector.tensor_scalar(zl[:], t2[:], ccol(C_GPRE + ot), ccol(C_BPRE + ot),
                                        op0=OP.mult, op1=OP.add)
                zln.append(zl)
                zb = zlbp.tile([128, 512], BF16, tag="zlb")
                nc.vector.tensor_copy(zb[:], zl[:])
                zlb.append(zb)
            h1 = []
            for ot in range(2 * CT):
                ps = tproj.tile([128, 512], F32, tag="tproj")
                for ci in range(CT):
                    nc.tensor.matmul(ps[:], lhsT=w1[ci][:, 128 * ot:128 * (ot + 1)],
                                     rhs=zlb[ci][:], start=(ci == 0), stop=(ci == CT - 1))
                h = h1p.tile([128, 512], BF16, tag="h1")
                nc.scalar.activation(h[:], ps[:], AF.Gelu, bias=ccol(C_B1 + ot))
                h1.append(h)
            res = []
            for ot in range(CT):
                ps = tproj.tile([128, 512], F32, tag="tproj")
                for ci in range(2 * CT):
                    nc.tensor.matmul(ps[:], lhsT=w2[ci][:, 128 * ot:128 * (ot + 1)],
                                     rhs=h1[ci][:], start=(ci == 0), stop=(ci == 2 * CT - 1))
                t1 = lntp.tile([128, 512], F32, tag="lnt")
                nc.vector.tensor_scalar_add(t1[:], ps[:], ccol(C_B2 + ot))
                r = resp.tile([128, 512], F32R, tag="res")
                nc.vector.tensor_tensor(out=r[:], in0=t1[:], in1=zln[ot][:], op=OP.add)
                res.append(r)
            mu_bc2, rstd_bc2 = tail_stats(res)
            for ot in range(CT):
                t1 = lntp.tile([128, 512], F32, tag="lnt")
                nc.vector.tensor_tensor(out=t1[:], in0=res[ot][:], in1=mu_bc2[:], op=OP.subtract)
                t2 = lntp.tile([128, 512], F32, tag="lnt2")
                nc.vector.tensor_tensor(out=t2[:], in0=t1[:], in1=rstd_bc2[:], op=OP.mult)
                o = outp.tile([128, 512], F32, tag="o")
                nc.vector.tensor_scalar(o[:], t2[:], ccol(C_GPOST + ot), ccol(C_BPOST + ot),
                                        op0=OP.mult, op1=OP.add)
                nc.sync.dma_start(out=d["out"][128 * ot:128 * (ot + 1), 512 * qb:512 * (qb + 1)], in_=o[:])

        p2.close()

    nc.compile()
    return nc


def _fold_ln(g, beta, W, b, extra_scale=1.0):
    g = np.asarray(g, np.float64); beta = np.asarray(beta, np.float64)
    W = np.asarray(W, np.float64); b = np.asarray(b, np.float64)
    gW = g[:, None] * W
    s = gW.sum(axis=0)
    Wpp = gW - s[None, :] / DIM
    bpp = b + beta @ W
    return (Wpp * extra_scale).astype(np.float32), (bpp * extra_scale).astype(np.float32)


def _cols(vec):
    return np.ascontiguousarray(np.asarray(vec, np.float32).reshape(-1, 128).T)


def kernel(q, k, v,
           ln_q_g, ln_q_b, ln_k_g, ln_k_b, ln_v_g, ln_v_b,
           Wq, bq, Wk, bk, Wv, bv, Wp, bp,
           ln_pre_g, ln_pre_b, W1, b1, W2, b2, ln_post_g, ln_post_b):
    import ml_dtypes
    q = np.asarray(q); k = np.asarray(k); v = np.asarray(v)
    qT = np.ascontiguousarray(q.reshape(DIM, QTOK).astype(np.float32))
    kT = np.ascontiguousarray(k.reshape(DIM, KTOK).astype(np.float32))
    vT = np.ascontiguousarray(v.reshape(DIM, KTOK).astype(np.float32))

    scale = DH ** -0.5
    Wq_f, bq_f = _fold_ln(ln_q_g, ln_q_b, Wq, bq, extra_scale=scale)
    Wk_f, bk_f = _fold_ln(ln_k_g, ln_k_b, Wk, bk)
    Wv_f, bv_f = _fold_ln(ln_v_g, ln_v_b, Wv, bv)

    cpack = np.zeros((128, 41), np.float32)
    cpack[:, 0:4] = _cols(bq_f)
    cpack[:, 4:8] = _cols(bk_f)
    cpack[:, 8:12] = _cols(bp)
    cpack[:, 12:20] = _cols(b1)
    cpack[:, 20:24] = _cols(b2)
    cpack[:, 24:28] = _cols(ln_pre_g)
    cpack[:, 28:32] = _cols(ln_pre_b)
    cpack[:, 32:36] = _cols(ln_post_g)
    cpack[:, 36:40] = _cols(ln_post_b)
    cpack[:, 40] = EPS

    common = {
        "kT": kT, "vT": vT,
        "Wq": Wq_f, "Wk": Wk_f, "Wv": Wv_f,
        "Wp": np.ascontiguousarray(np.asarray(Wp, np.float32)),
        "W1": np.ascontiguousarray(np.asarray(W1, np.float32)).astype(ml_dtypes.bfloat16),
        "W2": np.ascontiguousarray(np.asarray(W2, np.float32)).astype(ml_dtypes.bfloat16),
        "cpack": cpack,
        "bvr": bv_f.reshape(1, DIM),
        "vones": np.ones((1, HEADS), ml_dtypes.bfloat16),
        "onesc": np.ones((128, 1), np.float32),
        "onesr": np.ones((1, 128), np.float32),
    }
    in_maps = []
    for c in range(NCORES):
        m = dict(common)
        m["qT"] = np.ascontiguousarray(qT[:, QPC * c:QPC * (c + 1)])
        in_maps.append(m)

    if "nc" not in _CACHE:
        _CACHE["nc"] = _build_program()
    nc = _CACHE["nc"]

    res = run_bass_kernel_spmd(nc, in_maps, core_ids=list(range(NCORES)),
                               **_CACHE.get("run_kwargs", {}))
    _CACHE["last_results"] = res
    outT = np.concatenate([res.results[c]["out"] for c in range(NCORES)], axis=1)  # (512, 8192)
    return outT.reshape(1, DIM, 32, 32, 8).astype(np.float32)

